# revision 1
# baseline (speedup 1.0000x reference)
"""Distributed 2-layer GAT on 8 TRN2 NeuronCores via Bass/Tile.

Strategy (per core r of 8; nodes sharded contiguously, 6250/core):
  P0: h1 shard compute  h1aug = X_r @ W0aug  -> T1 shard rows
      [h1 bf16 (256) | s_dst1 f32x8 (16 bf16 slots) | pad] = 288 bf16 cols
  AG1: AllGather T1 shards -> T1full [50000, 288] in every core's HBM
  P2: per 128-node tile (2048 edges): 2-pass indirect row gather
      (int32 idx, oob-skip splits rows <32768 / >=32768), edge scores
      e = leaky(s_src + s_dst), p = exp(e), denominators via mask matmul,
      aggregation via PE (stationary = gathered rows, moving = att*mask),
      ELU -> X1T (transposed, SBUF-resident)
  P3: h2aug = X1 @ W1aug -> T2 shard rows [h2 bf16(64)|s2 f32x2(4 slots)|pad]=80
  AG2: AllGather T2 -> T2full [50000, 80]
  P4: layer-2 edge phase (single head), ELU, transpose -> out [6250, 64] f32
Host: index prep, weight folding (a-vectors into W), X transpose, concat.
"""

import numpy as np

import sys

if "/opt/trn_rl_repo" not in sys.path:
    sys.path.insert(0, "/opt/trn_rl_repo")

import ml_dtypes
from concourse import bass, tile, mybir
from concourse.bass import IndirectOffsetOnAxis
from concourse.bass_utils import run_bass_kernel_spmd

BF16 = mybir.dt.bfloat16
F32 = mybir.dt.float32
I32 = mybir.dt.int32
AX = mybir.AluOpType

N = 50000
DEG = 16
IN = 256
H = 8
DH = 32
HID = 256
OUT = 64
ALPHA = 0.2
NCORES = 8
NPC = N // NCORES            # 6250 nodes per core
SPLIT = 32768                # row-index split for int-offset tables
ROW1 = 288                   # T1 row cols (bf16): 256 h1 | 16 (8xf32 s_dst) | 16 pad
ROW2 = 80                    # T2 row cols (bf16): 64 h2 | 4 (2xf32? no: s_src2,s_dst2 f32) | pad
OOB = 1 << 30


def _split_excess_waits(nc, max_waits=1):
    """walrus in this container rejects >1 sync-wait per instruction; move
    extras onto preceding same-engine NOPs."""
    n_split = 0
    for bb in nc.main_func.blocks:
        new_list = []
        for ins in bb.instructions:
            si = ins.sync_info
            if si is not None and si.on_wait is not None and len(si.on_wait) > max_waits:
                waits = list(si.on_wait)
                extra, keep = waits[:-max_waits], waits[-max_waits:]
                while extra:
                    chunk, extra = extra[:max_waits], extra[max_waits:]
                    nop = mybir.InstNoOp(
                        name=nc.get_next_instruction_name(), ins=[], outs=[]
                    )
                    nop.engine = ins.engine
                    nop.sync_info = mybir.SyncInfo(on_wait=chunk, on_update=[])
                    new_list.append(nop)
                    n_split += 1
                ins.sync_info = mybir.SyncInfo(
                    on_wait=keep, on_update=list(si.on_update)
                )
            new_list.append(ins)
        bb.instructions[:] = new_list
    return n_split


def build_nc(ntiles, debug=False):
    """Build the SPMD Bass program. ntiles = node tiles per core (49 full)."""
    npad = ntiles * 128
    nval = min(NPC, npad)  # valid (unpadded) node count per core

    nc = bass.Bass("TRN2", target_bir_lowering=False, debug=False,
                   num_devices=NCORES)

    # ---- parameters (per-core values supplied via in_maps) ----
    xt = nc.declare_dram_parameter("xt", [2, 128, npad], BF16, isOutput=False)
    idx_lo = nc.declare_dram_parameter("idx_lo", [128, ntiles * 16], I32, isOutput=False)
    w0aug = nc.declare_dram_parameter("w0aug", [2, 128, 272], BF16, isOutput=False)
    w1aug = nc.declare_dram_parameter("w1aug", [2, 128, 66], BF16, isOutput=False)
    m2c = nc.declare_dram_parameter("m2", [128, 128], BF16, isOutput=False)
    mfixc = nc.declare_dram_parameter("mfix", [128, 8], BF16, isOutput=False)
    identc = nc.declare_dram_parameter("ident", [128, 128], F32, isOutput=False)
    out_p = nc.declare_dram_parameter("out", [nval, OUT], F32, isOutput=True)
    if debug:
        dbg_t1s = nc.declare_dram_parameter("dbg_t1s", [128, ROW1], BF16, isOutput=True)
        dbg_t1f = nc.declare_dram_parameter("dbg_t1f", [256, ROW1], BF16, isOutput=True)
        dbg_g = nc.declare_dram_parameter("dbg_g", [128, 16 * ROW1], BF16, isOutput=True)
        dbg_x1t = nc.declare_dram_parameter("dbg_x1t", [128, 2, 128], BF16, isOutput=True)

    with tile.TileContext(nc, num_cores=NCORES) as tc:
        with (
            tc.tile_pool(name="dram", bufs=1, space="DRAM") as dpool,
            tc.tile_pool(name="consts", bufs=1) as cpool,
            tc.tile_pool(name="persist", bufs=1) as ppool,
        ):
            # ---- DRAM scratch ----
            t1_shard = dpool.tile([nval, ROW1], BF16)
            t1_full = dpool.tile([N, ROW1], BF16, addr_space="Shared")
            t2_shard = dpool.tile([nval, ROW2], BF16)
            t2_full = dpool.tile([N, ROW2], BF16, addr_space="Shared")
            ssr1 = dpool.tile([npad * 16, 8], F32)    # s_src1 replicated x16
            ssr2 = dpool.tile([npad * 16, 1], F32)    # s_src2 replicated x16

            # ---- constants / persistent SBUF ----
            w0_sb = cpool.tile([128, 2, 272], BF16)
            nc.sync.dma_start(out=w0_sb[:], in_=w0aug[:].rearrange("c p f -> p c f"))
            w1_sb = cpool.tile([128, 2, 66], BF16)
            nc.sync.dma_start(out=w1_sb[:], in_=w1aug[:].rearrange("c p f -> p c f"))
            m2_sb = cpool.tile([128, 128], BF16)
            nc.sync.dma_start(out=m2_sb[:], in_=m2c[:])
            mfix_sb = cpool.tile([128, 8], BF16)
            nc.sync.dma_start(out=mfix_sb[:], in_=mfixc[:])
            id_sb = cpool.tile([128, 128], F32)
            nc.sync.dma_start(out=id_sb[:], in_=identc[:])
            il_sb = cpool.tile([128, ntiles * 16], I32)
            nc.sync.dma_start(out=il_sb[:], in_=idx_lo[:])

            x1t = ppool.tile([128, 2, npad], BF16)          # X1 transposed
            ss1_sb = ppool.tile([128, ntiles, 8], F32)      # s_src1 per node
            ss2_sb = ppool.tile([128, ntiles, 1], F32)      # s_src2 per node

            # ================= P0: layer-1 linear =================
            with (
                tc.tile_pool(name="p0s", bufs=3) as sp,
                tc.tile_pool(name="p0p", bufs=2, space="PSUM") as pp,
            ):
                for t in range(ntiles):
                    xtile = sp.tile([128, 2, 128], BF16, tag="xt")
                    nc.sync.dma_start(
                        out=xtile[:], in_=xt[:, :, t * 128:(t + 1) * 128]
                        .rearrange("c p n -> p c n"))
                    h1ps = pp.tile([128, 272], F32, tag="h1")
                    for ch in range(2):
                        nc.tensor.matmul(
                            out=h1ps[:], lhsT=xtile[:, ch, :], rhs=w0_sb[:, ch, :],
                            start=(ch == 0), stop=(ch == 1))
                    t1t = sp.tile([128, ROW1], BF16, tag="t1t")
                    nc.vector.tensor_copy(out=t1t[:, 0:256], in_=h1ps[:, 0:256])
                    nc.vector.tensor_copy(
                        out=t1t[:, 256:272].bitcast(F32), in_=h1ps[:, 264:272])
                    nc.vector.tensor_copy(out=ss1_sb[:, t, :], in_=h1ps[:, 256:264])
                    rows = min(128, nval - t * 128)
                    nc.sync.dma_start(
                        out=t1_shard[t * 128: t * 128 + rows, :], in_=t1t[:rows, :])

            # s_src1 -> replicated table (one DMA per k)
            for k in range(DEG):
                nc.sync.dma_start(
                    out=ssr1[:].rearrange("(n k) c -> n k c", k=16)[:, k, :]
                    .rearrange("(t p) c -> p t c", p=128),
                    in_=ss1_sb[:])

            # ================= AG1 =================
            nc.gpsimd.collective_compute(
                "AllGather", AX.bypass,
                replica_groups=[list(range(NCORES))],
                ins=[t1_shard[:]], outs=[t1_full[0:NCORES * nval]])

            # ================= P2: layer-1 edge phase =================
            with (
                tc.tile_pool(name="p2s", bufs=3) as sp,
                tc.tile_pool(name="p2ps", bufs=2) as sp2,
                tc.tile_pool(name="p2p", bufs=1, space="PSUM") as pp1,
                tc.tile_pool(name="p2pd", bufs=2, space="PSUM") as ppd,
            ):
                for t in range(ntiles):
                    g = sp.tile([128, 16, ROW1], BF16, tag="g1")
                    for b in range(16):
                        nc.gpsimd.indirect_dma_start(
                            out=g[:, b, :], out_offset=None, in_=t1_full[:],
                            in_offset=IndirectOffsetOnAxis(
                                ap=il_sb[:, t * 16 + b: t * 16 + b + 1], axis=0))
                    sse = sp2.tile([128, 16, 8], F32, tag="sse1")
                    nc.sync.dma_start(
                        out=sse[:],
                        in_=ssr1[:].rearrange("(t b p) c -> t p b c", b=16, p=128)[t])
                    # e = leaky(s_src + s_dst)
                    ef = sp2.tile([128, 16, 8], F32, tag="ef")
                    nc.vector.tensor_tensor(
                        out=ef[:], in0=sse[:], in1=g[:, :, 256:272].bitcast(F32),
                        op=AX.add)
                    el = sp2.tile([128, 16, 8], F32, tag="el")
                    nc.vector.scalar_tensor_tensor(
                        out=el[:], in0=ef[:], scalar=ALPHA, in1=ef[:],
                        op0=AX.mult, op1=AX.max)
                    pbf = sp2.tile([128, 16, 8], BF16, tag="pbf")
                    nc.scalar.activation(
                        out=pbf[:], in_=el[:],
                        func=mybir.ActivationFunctionType.Exp)
                    # denominators (per 16-edge group) + reciprocal
                    dn = ppd.tile([128, 128], F32, tag="dn")
                    nc.tensor.matmul(
                        out=dn[:], lhsT=m2_sb[:],
                        rhs=pbf[:].rearrange("p b h -> p (b h)"),
                        start=True, stop=True)
                    rcf = sp2.tile([128, 128], F32, tag="rcf")
                    nc.vector.reciprocal(out=rcf[:], in_=dn[:])
                    rcb = sp2.tile([128, 16, 8], BF16, tag="rcb")
                    nc.vector.tensor_copy(
                        out=rcb[:], in_=rcf[:].rearrange("p (b h) -> p b h", h=8))
                    qbf = sp2.tile([128, 16, 8], BF16, tag="qbf")
                    nc.vector.tensor_tensor(
                        out=qbf[:], in0=pbf[:], in1=rcb[:], op=AX.mult)
                    # S[e,(b,n,h)] = q[e,(b,h)] * mask[e,n]
                    sall = sp2.tile([128, 16, 8, 8], BF16, tag="sall")
                    nc.vector.tensor_tensor(
                        out=sall[:],
                        in0=qbf[:].unsqueeze(2).to_broadcast([128, 16, 8, 8]),
                        in1=mfix_sb[:].unsqueeze(1).unsqueeze(3)
                        .to_broadcast([128, 16, 8, 8]),
                        op=AX.mult)
                    # aggregation: psum[c, (b, n, h)] += G[e, c] * S[e, (b,n,h)]
                    aps = [pp1.tile([128, 16, 64], F32, tag=f"agg{ch}",
                                    name=f"agg{ch}_{t}") for ch in range(2)]
                    for ch in range(2):
                        for b in range(16):
                            nc.tensor.matmul(
                                out=aps[ch][:, b, :],
                                lhsT=g[:, b, ch * 128:(ch + 1) * 128],
                                rhs=sall[:, b, :, :].rearrange("p n h -> p (n h)"),
                                start=True, stop=True)
                    # extract X1T[c, n] = psum[c, (b, n, h(c))], ELU
                    for ch in range(2):
                        xr = sp2.tile([128, 128], F32, tag="xr")
                        for gq in range(4):
                            hh = ch * 4 + gq
                            nc.vector.tensor_copy(
                                out=xr[32 * gq:32 * (gq + 1), :],
                                in_=aps[ch][32 * gq:32 * (gq + 1), :, :]
                                .rearrange("p b (n h) -> p b n h", h=8)[:, :, :, hh]
                                .rearrange("p b n -> p (b n)"))
                        ex = sp2.tile([128, 128], F32, tag="ex")
                        nc.scalar.activation(
                            out=ex[:], in_=xr[:],
                            func=mybir.ActivationFunctionType.Exp)
                        ev = sp2.tile([128, 128], F32, tag="ev")
                        nc.vector.tensor_scalar(
                            out=ev[:], in0=ex[:], scalar1=-1.0, scalar2=0.0,
                            op0=AX.add, op1=AX.min)
                        nc.vector.scalar_tensor_tensor(
                            out=x1t[:, ch, t * 128:(t + 1) * 128],
                            in0=xr[:], scalar=0.0, in1=ev[:],
                            op0=AX.max, op1=AX.add)

            if debug:
                nc.sync.dma_start(out=dbg_t1s[:], in_=t1_shard[0:128, :])
                nc.sync.dma_start(out=dbg_t1f[:], in_=t1_full[0:256, :])
                dbg_g_sb = ppool.tile([128, 16, ROW1], BF16)
                for b in range(16):
                    nc.gpsimd.indirect_dma_start(
                        out=dbg_g_sb[:, b, :], out_offset=None, in_=t1_full[:],
                        in_offset=IndirectOffsetOnAxis(ap=il_sb[:, b:b + 1], axis=0))
                nc.sync.dma_start(
                    out=dbg_g[:], in_=dbg_g_sb[:].rearrange("p b c -> p (b c)"))
                nc.sync.dma_start(out=dbg_x1t[:], in_=x1t[:, :, 0:128])

            # ================= P3: layer-2 linear =================
            with (
                tc.tile_pool(name="p3s", bufs=3) as sp,
                tc.tile_pool(name="p3p", bufs=2, space="PSUM") as pp,
            ):
                for t in range(ntiles):
                    h2ps = pp.tile([66, 128], F32, tag="h2")
                    for ch in range(2):
                        nc.tensor.matmul(
                            out=h2ps[:], lhsT=w1_sb[:, ch, :],
                            rhs=x1t[:, ch, t * 128:(t + 1) * 128],
                            start=(ch == 0), stop=(ch == 1))
                    h2sb = sp.tile([66, 128], F32, tag="h2sb")
                    nc.vector.tensor_copy(out=h2sb[:], in_=h2ps[:])
                    tps = pp.tile([128, 66], F32, tag="tps")
                    nc.tensor.transpose(
                        out=tps[:], in_=h2sb[:], identity=id_sb[:66, :66])
                    t2t = sp.tile([128, ROW2], BF16, tag="t2t")
                    nc.vector.tensor_copy(out=t2t[:, 0:64], in_=tps[:, 0:64])
                    nc.vector.tensor_copy(
                        out=t2t[:, 64:66].bitcast(F32), in_=tps[:, 65:66])
                    nc.vector.tensor_copy(out=ss2_sb[:, t, :], in_=tps[:, 64:65])
                    rows = min(128, nval - t * 128)
                    nc.sync.dma_start(
                        out=t2_shard[t * 128: t * 128 + rows, :], in_=t2t[:rows, :])

            for k in range(DEG):
                nc.sync.dma_start(
                    out=ssr2[:].rearrange("(n k) c -> n k c", k=16)[:, k, :]
                    .rearrange("(t p) c -> p t c", p=128),
                    in_=ss2_sb[:])

            # ================= AG2 =================
            nc.gpsimd.collective_compute(
                "AllGather", AX.bypass,
                replica_groups=[list(range(NCORES))],
                ins=[t2_shard[:]], outs=[t2_full[0:NCORES * nval]])

            # ================= P4: layer-2 edge phase =================
            with (
                tc.tile_pool(name="p4s", bufs=3) as sp,
                tc.tile_pool(name="p4ps", bufs=2) as sp2,
                tc.tile_pool(name="p4p", bufs=2, space="PSUM") as pp,
            ):
                for t in range(ntiles):
                    g2 = sp.tile([128, 16, ROW2], BF16, tag="g2")
                    for b in range(16):
                        nc.gpsimd.indirect_dma_start(
                            out=g2[:, b, :], out_offset=None, in_=t2_full[:],
                            in_offset=IndirectOffsetOnAxis(
                                ap=il_sb[:, t * 16 + b: t * 16 + b + 1], axis=0))
                    sse2 = sp2.tile([128, 16, 1], F32, tag="sse2")
                    nc.sync.dma_start(
                        out=sse2[:],
                        in_=ssr2[:].rearrange("(t b p) c -> t p b c", b=16, p=128)[t])
                    ef2 = sp2.tile([128, 16], F32, tag="ef2")
                    nc.vector.tensor_tensor(
                        out=ef2[:], in0=sse2[:, :, 0],
                        in1=g2[:, :, 64:66].bitcast(F32)[:, :, 0], op=AX.add)
                    el2 = sp2.tile([128, 16], F32, tag="el2")
                    nc.vector.scalar_tensor_tensor(
                        out=el2[:], in0=ef2[:], scalar=ALPHA, in1=ef2[:],
                        op0=AX.mult, op1=AX.max)
                    p2b = sp2.tile([128, 16], BF16, tag="p2b")
                    nc.scalar.activation(
                        out=p2b[:], in_=el2[:],
                        func=mybir.ActivationFunctionType.Exp)
                    dn2 = pp.tile([128, 16], F32, tag="dn2")
                    nc.tensor.matmul(out=dn2[:], lhsT=m2_sb[:], rhs=p2b[:],
                                     start=True, stop=True)
                    rc2 = sp2.tile([128, 16], F32, tag="rc2")
                    nc.vector.reciprocal(out=rc2[:], in_=dn2[:])
                    rc2b = sp2.tile([128, 16], BF16, tag="rc2b")
                    nc.vector.tensor_copy(out=rc2b[:], in_=rc2[:])
                    q2b = sp2.tile([128, 16], BF16, tag="q2b")
                    nc.vector.tensor_tensor(
                        out=q2b[:], in0=p2b[:], in1=rc2b[:], op=AX.mult)
                    s2 = sp2.tile([128, 16, 8], BF16, tag="s2")
                    nc.vector.tensor_tensor(
                        out=s2[:],
                        in0=q2b[:].unsqueeze(2).to_broadcast([128, 16, 8]),
                        in1=mfix_sb[:].unsqueeze(1).to_broadcast([128, 16, 8]),
                        op=AX.mult)
                    o2ps = pp.tile([64, 16, 8], F32, tag="o2")
                    for b in range(16):
                        nc.tensor.matmul(
                            out=o2ps[:, b, :], lhsT=g2[:, b, 0:64],
                            rhs=s2[:, b, :], start=True, stop=True)
                    # ELU
                    ex2 = sp2.tile([64, 128], F32, tag="ex2")
                    nc.scalar.activation(
                        out=ex2[:], in_=o2ps[:].rearrange("p b n -> p (b n)"),
                        func=mybir.ActivationFunctionType.Exp)
                    ev2 = sp2.tile([64, 128], F32, tag="ev2")
                    nc.vector.tensor_scalar(
                        out=ev2[:], in0=ex2[:], scalar1=-1.0, scalar2=0.0,
                        op0=AX.add, op1=AX.min)
                    o2sb = sp2.tile([64, 128], F32, tag="o2sb")
                    nc.vector.scalar_tensor_tensor(
                        out=o2sb[:], in0=o2ps[:].rearrange("p b n -> p (b n)"),
                        scalar=0.0, in1=ev2[:], op0=AX.max, op1=AX.add)
                    ops_ = pp.tile([128, 64], F32, tag="otp")
                    nc.tensor.transpose(
                        out=ops_[:], in_=o2sb[:], identity=id_sb[:64, :64])
                    otile = sp.tile([128, 64], F32, tag="otile")
                    nc.vector.tensor_copy(out=otile[:], in_=ops_[:])
                    rows = min(128, nval - t * 128)
                    nc.sync.dma_start(
                        out=out_p[t * 128: t * 128 + rows, :], in_=otile[:rows, :])

    _split_excess_waits(nc)
    return nc


# ---------------- host-side preparation ----------------

def host_prep(X, edges, W0, a0, W1, a1, ntiles):
    npad = ntiles * 128
    nval = min(NPC, npad)
    f32 = np.float32

    W0flat = np.ascontiguousarray(W0.transpose(1, 0, 2)).reshape(IN, H * DH)
    v_src = np.stack([W0[h_] @ a0[h_, :DH] for h_ in range(H)], axis=1)   # [IN, 8]
    v_dst = np.stack([W0[h_] @ a0[h_, DH:] for h_ in range(H)], axis=1)
    W0aug = np.concatenate([W0flat, v_src, v_dst], axis=1).astype(f32)    # [256, 272]
    W1flat = W1[0]
    w1s = W1flat @ a1[0, :OUT]
    w1d = W1flat @ a1[0, OUT:]
    W1aug = np.concatenate([W1flat, w1s[:, None], w1d[:, None]], axis=1).astype(f32)

    w0aug_b = W0aug.reshape(2, 128, 272).astype(ml_dtypes.bfloat16)
    w1aug_b = W1aug.reshape(2, 128, 66).astype(ml_dtypes.bfloat16)

    q = np.arange(128)
    m2 = (q[:, None] // 16 == q[None, :] // 16).astype(ml_dtypes.bfloat16)
    mfix = (q[:, None] // 16 == np.arange(8)[None, :]).astype(ml_dtypes.bfloat16)
    ident = np.eye(128, dtype=np.float32)

    dst = np.asarray(edges[1], dtype=np.int64).reshape(N, DEG)
    ntot = NCORES * nval
    if ntot < N:  # small-mode structure test: remap edges into the node subset
        dst = dst % ntot

    in_maps = []
    for r in range(NCORES):
        xs = X[r * nval: (r + 1) * nval].astype(ml_dtypes.bfloat16)
        xt = np.zeros((256, npad), dtype=ml_dtypes.bfloat16)
        xt[:, :nval] = xs.T
        xtc = np.ascontiguousarray(xt.reshape(2, 128, npad))

        d = np.zeros((npad, DEG), dtype=np.int64)
        d[:nval] = dst[r * nval: (r + 1) * nval]
        # slot (p, b) of tile t -> node t*128 + 8*b + p//16, k = p%16
        tt, pp_, bb = np.meshgrid(np.arange(ntiles), np.arange(128),
                                  np.arange(16), indexing="ij")
        nloc = tt * 128 + bb * 8 + pp_ // 16
        kk = pp_ % 16
        dv = d[nloc, kk]                         # [ntiles, 128, 16]
        ilo = np.ascontiguousarray(
            dv.astype(np.int32).transpose(1, 0, 2)).reshape(128, ntiles * 16)

        in_maps.append({
            "xt": xtc, "idx_lo": ilo,
            "w0aug": w0aug_b, "w1aug": w1aug_b,
            "m2": m2, "mfix": mfix, "ident": ident,
        })
    return in_maps


_CACHE = {}


def run_gat(X, edges, W0, a0, W1, a1, ntiles=(NPC + 127) // 128, **run_kwargs):
    X = np.asarray(X, dtype=np.float32)
    edges = np.asarray(edges)
    in_maps = host_prep(X, edges, np.asarray(W0, np.float32),
                        np.asarray(a0, np.float32), np.asarray(W1, np.float32),
                        np.asarray(a1, np.float32), ntiles)
    debug = run_kwargs.pop("debug", False)
    key = (ntiles, debug)
    if key not in _CACHE:
        _CACHE[key] = build_nc(ntiles, debug=debug)
    nc = _CACHE[key]
    res = run_bass_kernel_spmd(nc, in_maps, core_ids=list(range(NCORES)),
                               **run_kwargs)
    nval = min(NPC, ntiles * 128)
    out = np.concatenate([res.results[r]["out"][:nval] for r in range(NCORES)], 0)
    return out, res


def kernel(X, edges, W0, a0, W1, a1):
    out, _ = run_gat(X, edges, W0, a0, W1, a1)
    return out.astype(np.float32)


# revision 2
# speedup vs baseline: 1.7200x; 1.7200x over previous
"""Distributed 2-layer GAT on 8 TRN2 NeuronCores via Bass/Tile.

Strategy (per core r of 8; nodes sharded contiguously, 6250/core):
  P0: h1 shard compute  h1aug = X_r @ W0aug  -> T1 shard rows
      [h1 bf16 (256) | s_dst1 f32x8 (16 bf16 slots) | pad] = 288 bf16 cols
  AG1: AllGather T1 shards -> T1full [50000, 288] in every core's HBM
  P2: per 128-node tile (2048 edges): 2-pass indirect row gather
      (int32 idx, oob-skip splits rows <32768 / >=32768), edge scores
      e = leaky(s_src + s_dst), p = exp(e), denominators via mask matmul,
      aggregation via PE (stationary = gathered rows, moving = att*mask),
      ELU -> X1T (transposed, SBUF-resident)
  P3: h2aug = X1 @ W1aug -> T2 shard rows [h2 bf16(64)|s2 f32x2(4 slots)|pad]=80
  AG2: AllGather T2 -> T2full [50000, 80]
  P4: layer-2 edge phase (single head), ELU, transpose -> out [6250, 64] f32
Host: index prep, weight folding (a-vectors into W), X transpose, concat.
"""

import numpy as np

import sys

if "/opt/trn_rl_repo" not in sys.path:
    sys.path.insert(0, "/opt/trn_rl_repo")

import ml_dtypes
from concourse import bass, tile, mybir
from concourse.bass import IndirectOffsetOnAxis
from concourse.bass_utils import run_bass_kernel_spmd

BF16 = mybir.dt.bfloat16
F32 = mybir.dt.float32
I32 = mybir.dt.int32
AX = mybir.AluOpType

N = 50000
DEG = 16
IN = 256
H = 8
DH = 32
HID = 256
OUT = 64
ALPHA = 0.2
NCORES = 8
NPC = N // NCORES            # 6250 nodes per core
SPLIT = 32768                # row-index split for int-offset tables
ROW1 = 288                   # T1 row cols (bf16): 256 h1 | 16 (8xf32 s_dst) | 16 pad
ROW2 = 80                    # T2 row cols (bf16): 64 h2 | 4 (2xf32? no: s_src2,s_dst2 f32) | pad
OOB = 1 << 30


def _split_excess_waits(nc, max_waits=1):
    """walrus in this container rejects >1 sync-wait per instruction; move
    extras onto preceding same-engine NOPs."""
    n_split = 0
    for bb in nc.main_func.blocks:
        new_list = []
        for ins in bb.instructions:
            si = ins.sync_info
            if si is not None and si.on_wait is not None and len(si.on_wait) > max_waits:
                waits = list(si.on_wait)
                extra, keep = waits[:-max_waits], waits[-max_waits:]
                while extra:
                    chunk, extra = extra[:max_waits], extra[max_waits:]
                    nop = mybir.InstNoOp(
                        name=nc.get_next_instruction_name(), ins=[], outs=[]
                    )
                    nop.engine = ins.engine
                    nop.sync_info = mybir.SyncInfo(on_wait=chunk, on_update=[])
                    new_list.append(nop)
                    n_split += 1
                ins.sync_info = mybir.SyncInfo(
                    on_wait=keep, on_update=list(si.on_update)
                )
            new_list.append(ins)
        bb.instructions[:] = new_list
    return n_split


def build_nc(ntiles, debug=False):
    """Build the SPMD Bass program. ntiles = node tiles per core (49 full)."""
    npad = ntiles * 128
    nval = min(NPC, npad)  # valid (unpadded) node count per core

    nc = bass.Bass("TRN2", target_bir_lowering=False, debug=False,
                   num_devices=NCORES, num_swdge_queues=4)

    # ---- parameters (per-core values supplied via in_maps) ----
    xt = nc.declare_dram_parameter("xt", [2, 128, npad], BF16, isOutput=False)
    idx_lo = nc.declare_dram_parameter("idx_lo", [128, ntiles * 16], I32, isOutput=False)
    w0aug = nc.declare_dram_parameter("w0aug", [2, 128, 272], BF16, isOutput=False)
    w1aug = nc.declare_dram_parameter("w1aug", [2, 128, 66], BF16, isOutput=False)
    m2c = nc.declare_dram_parameter("m2", [128, 128], BF16, isOutput=False)
    mfixc = nc.declare_dram_parameter("mfix", [128, 8], BF16, isOutput=False)
    identc = nc.declare_dram_parameter("ident", [128, 128], F32, isOutput=False)
    out_p = nc.declare_dram_parameter("out", [nval, OUT], F32, isOutput=True)
    if debug:
        dbg_t1s = nc.declare_dram_parameter("dbg_t1s", [128, ROW1], BF16, isOutput=True)
        dbg_t1f = nc.declare_dram_parameter("dbg_t1f", [256, ROW1], BF16, isOutput=True)
        dbg_g = nc.declare_dram_parameter("dbg_g", [128, 16 * ROW1], BF16, isOutput=True)
        dbg_x1t = nc.declare_dram_parameter("dbg_x1t", [128, 2, 128], BF16, isOutput=True)

    with tile.TileContext(nc, num_cores=NCORES) as tc:
        with (
            tc.tile_pool(name="dram", bufs=1, space="DRAM") as dpool,
            tc.tile_pool(name="consts", bufs=1) as cpool,
            tc.tile_pool(name="persist", bufs=1) as ppool,
        ):
            # ---- DRAM scratch ----
            t1_shard = dpool.tile([nval, ROW1], BF16)
            t1_full = dpool.tile([N, ROW1], BF16, addr_space="Shared")
            t2_shard = dpool.tile([nval, ROW2], BF16)
            t2_full = dpool.tile([N, ROW2], BF16, addr_space="Shared")
            ssr1 = dpool.tile([npad * 16, 8], F32)    # s_src1 replicated x16
            ssr2 = dpool.tile([npad * 16, 1], F32)    # s_src2 replicated x16

            # ---- constants / persistent SBUF ----
            w0_sb = cpool.tile([128, 2, 272], BF16)
            nc.sync.dma_start(out=w0_sb[:], in_=w0aug[:].rearrange("c p f -> p c f"))
            w1_sb = cpool.tile([128, 2, 66], BF16)
            nc.sync.dma_start(out=w1_sb[:], in_=w1aug[:].rearrange("c p f -> p c f"))
            m2_sb = cpool.tile([128, 128], BF16)
            nc.sync.dma_start(out=m2_sb[:], in_=m2c[:])
            mfix_sb = cpool.tile([128, 8], BF16)
            nc.sync.dma_start(out=mfix_sb[:], in_=mfixc[:])
            id_sb = cpool.tile([128, 128], F32)
            nc.sync.dma_start(out=id_sb[:], in_=identc[:])
            il_sb = cpool.tile([128, ntiles * 16], I32)
            nc.sync.dma_start(out=il_sb[:], in_=idx_lo[:])

            x1t = ppool.tile([128, 2, npad], BF16)          # X1 transposed
            ss1_sb = ppool.tile([128, ntiles, 8], F32)      # s_src1 per node
            ss2_sb = ppool.tile([128, ntiles, 1], F32)      # s_src2 per node

            # ================= P0: layer-1 linear =================
            with (
                tc.tile_pool(name="p0s", bufs=3) as sp,
                tc.tile_pool(name="p0p", bufs=2, space="PSUM") as pp,
            ):
                for t in range(ntiles):
                    xtile = sp.tile([128, 2, 128], BF16, tag="xt")
                    nc.sync.dma_start(
                        out=xtile[:], in_=xt[:, :, t * 128:(t + 1) * 128]
                        .rearrange("c p n -> p c n"))
                    h1ps = pp.tile([128, 272], F32, tag="h1")
                    for ch in range(2):
                        nc.tensor.matmul(
                            out=h1ps[:], lhsT=xtile[:, ch, :], rhs=w0_sb[:, ch, :],
                            start=(ch == 0), stop=(ch == 1))
                    t1t = sp.tile([128, ROW1], BF16, tag="t1t")
                    nc.vector.tensor_copy(out=t1t[:, 0:256], in_=h1ps[:, 0:256])
                    nc.vector.tensor_copy(
                        out=t1t[:, 256:272].bitcast(F32), in_=h1ps[:, 264:272])
                    nc.vector.tensor_copy(out=ss1_sb[:, t, :], in_=h1ps[:, 256:264])
                    rows = min(128, nval - t * 128)
                    nc.sync.dma_start(
                        out=t1_shard[t * 128: t * 128 + rows, :], in_=t1t[:rows, :])

            # s_src1 -> replicated table (one DMA per k)
            for k in range(DEG):
                nc.sync.dma_start(
                    out=ssr1[:].rearrange("(n k) c -> n k c", k=16)[:, k, :]
                    .rearrange("(t p) c -> p t c", p=128),
                    in_=ss1_sb[:])

            # ================= AG1 =================
            nc.gpsimd.collective_compute(
                "AllGather", AX.bypass,
                replica_groups=[list(range(NCORES))],
                ins=[t1_shard[:]], outs=[t1_full[0:NCORES * nval]])

            # ================= P2: layer-1 edge phase =================
            with (
                tc.tile_pool(name="p2s", bufs=3) as sp,
                tc.tile_pool(name="p2ps", bufs=2) as sp2,
                tc.tile_pool(name="p2p", bufs=1, space="PSUM") as pp1,
                tc.tile_pool(name="p2pd", bufs=2, space="PSUM") as ppd,
            ):
                for t in range(ntiles):
                    g = sp.tile([128, 16, ROW1], BF16, tag="g1")
                    for b in range(16):
                        gi = nc.gpsimd.indirect_dma_start(
                            out=g[:, b, :], out_offset=None, in_=t1_full[:],
                            in_offset=IndirectOffsetOnAxis(
                                ap=il_sb[:, t * 16 + b: t * 16 + b + 1], axis=0))
                        gi.ins.queue = f"qPoolDynamic{b % 4 or ''}"
                    sse = sp2.tile([128, 16, 8], F32, tag="sse1")
                    nc.sync.dma_start(
                        out=sse[:],
                        in_=ssr1[:].rearrange("(t b p) c -> t p b c", b=16, p=128)[t])
                    # e = leaky(s_src + s_dst)
                    ef = sp2.tile([128, 16, 8], F32, tag="ef")
                    nc.vector.tensor_tensor(
                        out=ef[:], in0=sse[:], in1=g[:, :, 256:272].bitcast(F32),
                        op=AX.add)
                    el = sp2.tile([128, 16, 8], F32, tag="el")
                    nc.vector.scalar_tensor_tensor(
                        out=el[:], in0=ef[:], scalar=ALPHA, in1=ef[:],
                        op0=AX.mult, op1=AX.max)
                    pbf = sp2.tile([128, 16, 8], BF16, tag="pbf")
                    nc.scalar.activation(
                        out=pbf[:], in_=el[:],
                        func=mybir.ActivationFunctionType.Exp)
                    # denominators (per 16-edge group) + reciprocal
                    dn = ppd.tile([128, 128], F32, tag="dn")
                    nc.tensor.matmul(
                        out=dn[:], lhsT=m2_sb[:],
                        rhs=pbf[:].rearrange("p b h -> p (b h)"),
                        start=True, stop=True)
                    rcf = sp2.tile([128, 128], F32, tag="rcf")
                    nc.vector.reciprocal(out=rcf[:], in_=dn[:])
                    rcb = sp2.tile([128, 16, 8], BF16, tag="rcb")
                    nc.vector.tensor_copy(
                        out=rcb[:], in_=rcf[:].rearrange("p (b h) -> p b h", h=8))
                    qbf = sp2.tile([128, 16, 8], BF16, tag="qbf")
                    nc.vector.tensor_tensor(
                        out=qbf[:], in0=pbf[:], in1=rcb[:], op=AX.mult)
                    # S[e,(b,n,h)] = q[e,(b,h)] * mask[e,n]
                    sall = sp2.tile([128, 16, 8, 8], BF16, tag="sall")
                    nc.vector.tensor_tensor(
                        out=sall[:],
                        in0=qbf[:].unsqueeze(2).to_broadcast([128, 16, 8, 8]),
                        in1=mfix_sb[:].unsqueeze(1).unsqueeze(3)
                        .to_broadcast([128, 16, 8, 8]),
                        op=AX.mult)
                    # aggregation: psum[c, (b, n, h)] += G[e, c] * S[e, (b,n,h)]
                    aps = [pp1.tile([128, 16, 64], F32, tag=f"agg{ch}",
                                    name=f"agg{ch}_{t}") for ch in range(2)]
                    for ch in range(2):
                        for b in range(16):
                            nc.tensor.matmul(
                                out=aps[ch][:, b, :],
                                lhsT=g[:, b, ch * 128:(ch + 1) * 128],
                                rhs=sall[:, b, :, :].rearrange("p n h -> p (n h)"),
                                start=True, stop=True)
                    # extract X1T[c, n] = psum[c, (b, n, h(c))], ELU
                    for ch in range(2):
                        xr = sp2.tile([128, 128], F32, tag="xr")
                        for gq in range(4):
                            hh = ch * 4 + gq
                            nc.vector.tensor_copy(
                                out=xr[32 * gq:32 * (gq + 1), :],
                                in_=aps[ch][32 * gq:32 * (gq + 1), :, :]
                                .rearrange("p b (n h) -> p b n h", h=8)[:, :, :, hh]
                                .rearrange("p b n -> p (b n)"))
                        ex = sp2.tile([128, 128], F32, tag="ex")
                        nc.scalar.activation(
                            out=ex[:], in_=xr[:],
                            func=mybir.ActivationFunctionType.Exp)
                        ev = sp2.tile([128, 128], F32, tag="ev")
                        nc.vector.tensor_scalar(
                            out=ev[:], in0=ex[:], scalar1=-1.0, scalar2=0.0,
                            op0=AX.add, op1=AX.min)
                        nc.vector.scalar_tensor_tensor(
                            out=x1t[:, ch, t * 128:(t + 1) * 128],
                            in0=xr[:], scalar=0.0, in1=ev[:],
                            op0=AX.max, op1=AX.add)

            if debug:
                nc.sync.dma_start(out=dbg_t1s[:], in_=t1_shard[0:128, :])
                nc.sync.dma_start(out=dbg_t1f[:], in_=t1_full[0:256, :])
                dbg_g_sb = ppool.tile([128, 16, ROW1], BF16)
                for b in range(16):
                    nc.gpsimd.indirect_dma_start(
                        out=dbg_g_sb[:, b, :], out_offset=None, in_=t1_full[:],
                        in_offset=IndirectOffsetOnAxis(ap=il_sb[:, b:b + 1], axis=0))
                nc.sync.dma_start(
                    out=dbg_g[:], in_=dbg_g_sb[:].rearrange("p b c -> p (b c)"))
                nc.sync.dma_start(out=dbg_x1t[:], in_=x1t[:, :, 0:128])

            # ================= P3: layer-2 linear =================
            with (
                tc.tile_pool(name="p3s", bufs=3) as sp,
                tc.tile_pool(name="p3p", bufs=2, space="PSUM") as pp,
            ):
                for t in range(ntiles):
                    h2ps = pp.tile([66, 128], F32, tag="h2")
                    for ch in range(2):
                        nc.tensor.matmul(
                            out=h2ps[:], lhsT=w1_sb[:, ch, :],
                            rhs=x1t[:, ch, t * 128:(t + 1) * 128],
                            start=(ch == 0), stop=(ch == 1))
                    h2sb = sp.tile([66, 128], F32, tag="h2sb")
                    nc.vector.tensor_copy(out=h2sb[:], in_=h2ps[:])
                    tps = pp.tile([128, 66], F32, tag="tps")
                    nc.tensor.transpose(
                        out=tps[:], in_=h2sb[:], identity=id_sb[:66, :66])
                    t2t = sp.tile([128, ROW2], BF16, tag="t2t")
                    nc.vector.tensor_copy(out=t2t[:, 0:64], in_=tps[:, 0:64])
                    nc.vector.tensor_copy(
                        out=t2t[:, 64:66].bitcast(F32), in_=tps[:, 65:66])
                    nc.vector.tensor_copy(out=ss2_sb[:, t, :], in_=tps[:, 64:65])
                    rows = min(128, nval - t * 128)
                    nc.sync.dma_start(
                        out=t2_shard[t * 128: t * 128 + rows, :], in_=t2t[:rows, :])

            for k in range(DEG):
                nc.sync.dma_start(
                    out=ssr2[:].rearrange("(n k) c -> n k c", k=16)[:, k, :]
                    .rearrange("(t p) c -> p t c", p=128),
                    in_=ss2_sb[:])

            # ================= AG2 =================
            nc.gpsimd.collective_compute(
                "AllGather", AX.bypass,
                replica_groups=[list(range(NCORES))],
                ins=[t2_shard[:]], outs=[t2_full[0:NCORES * nval]])

            # ================= P4: layer-2 edge phase =================
            with (
                tc.tile_pool(name="p4s", bufs=3) as sp,
                tc.tile_pool(name="p4ps", bufs=2) as sp2,
                tc.tile_pool(name="p4p", bufs=2, space="PSUM") as pp,
            ):
                for t in range(ntiles):
                    g2 = sp.tile([128, 16, ROW2], BF16, tag="g2")
                    for b in range(16):
                        gi2 = nc.gpsimd.indirect_dma_start(
                            out=g2[:, b, :], out_offset=None, in_=t2_full[:],
                            in_offset=IndirectOffsetOnAxis(
                                ap=il_sb[:, t * 16 + b: t * 16 + b + 1], axis=0))
                        gi2.ins.queue = f"qPoolDynamic{b % 4 or ''}"
                    sse2 = sp2.tile([128, 16, 1], F32, tag="sse2")
                    nc.sync.dma_start(
                        out=sse2[:],
                        in_=ssr2[:].rearrange("(t b p) c -> t p b c", b=16, p=128)[t])
                    ef2 = sp2.tile([128, 16], F32, tag="ef2")
                    nc.vector.tensor_tensor(
                        out=ef2[:], in0=sse2[:, :, 0],
                        in1=g2[:, :, 64:66].bitcast(F32)[:, :, 0], op=AX.add)
                    el2 = sp2.tile([128, 16], F32, tag="el2")
                    nc.vector.scalar_tensor_tensor(
                        out=el2[:], in0=ef2[:], scalar=ALPHA, in1=ef2[:],
                        op0=AX.mult, op1=AX.max)
                    p2b = sp2.tile([128, 16], BF16, tag="p2b")
                    nc.scalar.activation(
                        out=p2b[:], in_=el2[:],
                        func=mybir.ActivationFunctionType.Exp)
                    dn2 = pp.tile([128, 16], F32, tag="dn2")
                    nc.tensor.matmul(out=dn2[:], lhsT=m2_sb[:], rhs=p2b[:],
                                     start=True, stop=True)
                    rc2 = sp2.tile([128, 16], F32, tag="rc2")
                    nc.vector.reciprocal(out=rc2[:], in_=dn2[:])
                    rc2b = sp2.tile([128, 16], BF16, tag="rc2b")
                    nc.vector.tensor_copy(out=rc2b[:], in_=rc2[:])
                    q2b = sp2.tile([128, 16], BF16, tag="q2b")
                    nc.vector.tensor_tensor(
                        out=q2b[:], in0=p2b[:], in1=rc2b[:], op=AX.mult)
                    s2 = sp2.tile([128, 16, 8], BF16, tag="s2")
                    nc.vector.tensor_tensor(
                        out=s2[:],
                        in0=q2b[:].unsqueeze(2).to_broadcast([128, 16, 8]),
                        in1=mfix_sb[:].unsqueeze(1).to_broadcast([128, 16, 8]),
                        op=AX.mult)
                    o2ps = pp.tile([64, 16, 8], F32, tag="o2")
                    for b in range(16):
                        nc.tensor.matmul(
                            out=o2ps[:, b, :], lhsT=g2[:, b, 0:64],
                            rhs=s2[:, b, :], start=True, stop=True)
                    # ELU
                    ex2 = sp2.tile([64, 128], F32, tag="ex2")
                    nc.scalar.activation(
                        out=ex2[:], in_=o2ps[:].rearrange("p b n -> p (b n)"),
                        func=mybir.ActivationFunctionType.Exp)
                    ev2 = sp2.tile([64, 128], F32, tag="ev2")
                    nc.vector.tensor_scalar(
                        out=ev2[:], in0=ex2[:], scalar1=-1.0, scalar2=0.0,
                        op0=AX.add, op1=AX.min)
                    o2sb = sp2.tile([64, 128], F32, tag="o2sb")
                    nc.vector.scalar_tensor_tensor(
                        out=o2sb[:], in0=o2ps[:].rearrange("p b n -> p (b n)"),
                        scalar=0.0, in1=ev2[:], op0=AX.max, op1=AX.add)
                    ops_ = pp.tile([128, 64], F32, tag="otp")
                    nc.tensor.transpose(
                        out=ops_[:], in_=o2sb[:], identity=id_sb[:64, :64])
                    otile = sp.tile([128, 64], F32, tag="otile")
                    nc.vector.tensor_copy(out=otile[:], in_=ops_[:])
                    rows = min(128, nval - t * 128)
                    nc.sync.dma_start(
                        out=out_p[t * 128: t * 128 + rows, :], in_=otile[:rows, :])

    _split_excess_waits(nc)
    return nc


# ---------------- host-side preparation ----------------

def host_prep(X, edges, W0, a0, W1, a1, ntiles):
    npad = ntiles * 128
    nval = min(NPC, npad)
    f32 = np.float32

    W0flat = np.ascontiguousarray(W0.transpose(1, 0, 2)).reshape(IN, H * DH)
    v_src = np.stack([W0[h_] @ a0[h_, :DH] for h_ in range(H)], axis=1)   # [IN, 8]
    v_dst = np.stack([W0[h_] @ a0[h_, DH:] for h_ in range(H)], axis=1)
    W0aug = np.concatenate([W0flat, v_src, v_dst], axis=1).astype(f32)    # [256, 272]
    W1flat = W1[0]
    w1s = W1flat @ a1[0, :OUT]
    w1d = W1flat @ a1[0, OUT:]
    W1aug = np.concatenate([W1flat, w1s[:, None], w1d[:, None]], axis=1).astype(f32)

    w0aug_b = W0aug.reshape(2, 128, 272).astype(ml_dtypes.bfloat16)
    w1aug_b = W1aug.reshape(2, 128, 66).astype(ml_dtypes.bfloat16)

    q = np.arange(128)
    m2 = (q[:, None] // 16 == q[None, :] // 16).astype(ml_dtypes.bfloat16)
    mfix = (q[:, None] // 16 == np.arange(8)[None, :]).astype(ml_dtypes.bfloat16)
    ident = np.eye(128, dtype=np.float32)

    dst = np.asarray(edges[1], dtype=np.int64).reshape(N, DEG)
    ntot = NCORES * nval
    if ntot < N:  # small-mode structure test: remap edges into the node subset
        dst = dst % ntot

    in_maps = []
    for r in range(NCORES):
        xs = X[r * nval: (r + 1) * nval].astype(ml_dtypes.bfloat16)
        xt = np.zeros((256, npad), dtype=ml_dtypes.bfloat16)
        xt[:, :nval] = xs.T
        xtc = np.ascontiguousarray(xt.reshape(2, 128, npad))

        d = np.zeros((npad, DEG), dtype=np.int64)
        d[:nval] = dst[r * nval: (r + 1) * nval]
        # slot (p, b) of tile t -> node t*128 + 8*b + p//16, k = p%16
        tt, pp_, bb = np.meshgrid(np.arange(ntiles), np.arange(128),
                                  np.arange(16), indexing="ij")
        nloc = tt * 128 + bb * 8 + pp_ // 16
        kk = pp_ % 16
        dv = d[nloc, kk]                         # [ntiles, 128, 16]
        ilo = np.ascontiguousarray(
            dv.astype(np.int32).transpose(1, 0, 2)).reshape(128, ntiles * 16)

        in_maps.append({
            "xt": xtc, "idx_lo": ilo,
            "w0aug": w0aug_b, "w1aug": w1aug_b,
            "m2": m2, "mfix": mfix, "ident": ident,
        })
    return in_maps


_CACHE = {}


def run_gat(X, edges, W0, a0, W1, a1, ntiles=(NPC + 127) // 128, **run_kwargs):
    X = np.asarray(X, dtype=np.float32)
    edges = np.asarray(edges)
    in_maps = host_prep(X, edges, np.asarray(W0, np.float32),
                        np.asarray(a0, np.float32), np.asarray(W1, np.float32),
                        np.asarray(a1, np.float32), ntiles)
    debug = run_kwargs.pop("debug", False)
    key = (ntiles, debug)
    if key not in _CACHE:
        _CACHE[key] = build_nc(ntiles, debug=debug)
    nc = _CACHE[key]
    res = run_bass_kernel_spmd(nc, in_maps, core_ids=list(range(NCORES)),
                               **run_kwargs)
    nval = min(NPC, ntiles * 128)
    out = np.concatenate([res.results[r]["out"][:nval] for r in range(NCORES)], 0)
    return out, res


def kernel(X, edges, W0, a0, W1, a1):
    out, _ = run_gat(X, edges, W0, a0, W1, a1)
    return out.astype(np.float32)
